# revision 10
# baseline (speedup 1.0000x reference)
"""Trainium2 Bass kernel for the SelfAttentionBlock problem (8 NeuronCores).

Sharding strategy:
  * MLP (q/k/v two-layer GELU blocks): data-parallel over rows — each core
    owns 256 tokens of each batch (512 rows total) and computes full-width
    q/k/v for those rows. No weight-partial sums, no all-reduce.
  * Attention: head-parallel — core c computes head c for both batches.
    Rows->heads redistribution is a single AllToAll per tensor (q, k, v).
  * k/v/attention outputs are gathered and reassembled on the host (the
    host-side concat is the "all-gather" of the final [N,S,D] outputs).

Layouts (all chosen so every matmul operand is produced in the layout the
TensorEngine wants — no on-device transposes anywhere):
  * x is shipped pre-transposed per core: xT [D, ROWS].
  * layer-1 output is h1T = gelu(W1^T xT + b1) [H, ROWS]  (lhsT=W1 natural).
  * q/k layer-2 produce qT/kT [D, ROWS] (lhsT=W2 natural, rhs=h1T).
  * v layer-2 produces v [ROWS, D] natural (lhsT=h1T, rhs=W2), bias via a
    K=1 ones-row matmul, outer gelu on ACT.
  * attention scores are computed transposed: scT [k, q] (lhsT=kT, rhs=qT),
    exp on ACT (scale=1/8 fused), causal mask via 0/1 band-mask multiply on
    DVE (only diagonal tiles), and the PV matmul consumes exp'd scores
    directly as the moving operand: oT [65, q] (lhsT=v_aug) where v_aug has
    a ones column so row 64 accumulates the softmax denominator.
  * fully-masked score tiles (above the causal diagonal) are never computed.
Matmuls run in float32r (full PE rate at fp32 storage, ~1e-4 matmul rel err).
"""

import numpy as np

import concourse.bass as bass
import concourse.mybir as mybir
from concourse import bacc, tile
from concourse.bass_utils import run_bass_kernel_spmd

N_CORES = 8
N, S, D, H = 2, 2048, 512, 2048
HEADS = 8
Dh = D // HEADS            # 64
RPC = S // N_CORES         # 256 rows per core per batch
ROWS = N * RPC             # 512 rows per core
KT1 = D // 128             # 4 layer-1 contraction tiles
MT1 = H // 128             # 16 layer-1 out tiles == layer-2 contraction tiles
MT2 = D // 128             # 4 layer-2 out tiles
QC = S // 512              # 4 q-chunks per batch

F32 = mybir.dt.float32
F32R = mybir.dt.float32r
AF = mybir.ActivationFunctionType


def _build():
    nc = bacc.Bacc("TRN2", target_bir_lowering=False, debug=False,
                   num_devices=N_CORES)

    def din(name, shape):
        return nc.dram_tensor(name, shape, F32R, kind="ExternalInput")

    def dout(name, shape):
        return nc.dram_tensor(name, shape, F32R, kind="ExternalOutput")

    xT = din("xT", [D, ROWS])
    w1 = {t: din(f"w1{t}", [D, H]) for t in "kqv"}
    w2 = {t: din(f"w2{t}", [H, D]) for t in "kqv"}
    b1 = {t: din(f"b1{t}", [H]) for t in "kqv"}
    # b2 for k/q feeds DVE tensor_scalar_add, which requires plain float32
    b2 = {t: nc.dram_tensor(f"b2{t}", [D], F32 if t in "kq" else F32R,
                            kind="ExternalInput") for t in "kqv"}
    band_d = din("band", [128, 1024])
    ones_col_d = din("ones_col", [128, MT1])
    ones_row_d = din("ones_row", [1, 128])
    kT_out = dout("kT_out", [D, ROWS])
    v_out = dout("v_out", [ROWS, D])
    aT_out = dout("attn_outT", [N, Dh, S])

    with tile.TileContext(nc) as tc:
        with (
            tc.tile_pool(name="dram", bufs=1, space="DRAM") as dp,
            tc.tile_pool(name="cst", bufs=1) as cst,
            tc.tile_pool(name="w1p", bufs=5) as w1p,
            tc.tile_pool(name="w2p", bufs=16) as w2p,
            tc.tile_pool(name="h1p", bufs=18) as h1p,
            tc.tile_pool(name="l2p", bufs=4) as l2p,
            tc.tile_pool(name="att", bufs=1) as attp,
            tc.tile_pool(name="exp", bufs=6) as expp,
            tc.tile_pool(name="sm", bufs=2) as smp,
            tc.tile_pool(name="ps", bufs=4, space="PSUM") as psp,
            tc.tile_pool(name="pso", bufs=2, space="PSUM") as psop,
            tc.tile_pool(name="psb", bufs=2, space="PSUM") as psbp,
        ):
            send = {
                "k": dp.tile([D, ROWS], F32R, tag="send_k", name="send_k"),
                "q": dp.tile([D, ROWS], F32R, tag="send_q", name="send_q"),
                "v": dp.tile([N_CORES, ROWS, Dh], F32R, tag="send_v",
                             name="send_v"),
            }
            recv = {
                "k": dp.tile([D, ROWS], F32R, tag="recv_k", name="recv_k"),
                "q": dp.tile([D, ROWS], F32R, tag="recv_q", name="recv_q"),
                "v": dp.tile([N_CORES, ROWS, Dh], F32R, tag="recv_v",
                             name="recv_v"),
            }

            # ---- persistent tiles ----
            xt = cst.tile([128, KT1 * ROWS], F32R, tag="xt")
            for kt in range(KT1):
                nc.sync.dma_start(xt[:, kt * ROWS:(kt + 1) * ROWS],
                                  xT[kt * 128:(kt + 1) * 128, :])
            band_sb = cst.tile([128, 1024], F32R, tag="band")
            nc.sync.dma_start(band_sb[:], band_d[:])
            aux = cst.tile([1, 128 + D], F32R, tag="aux")
            nc.sync.dma_start(aux[:, 0:128], ones_row_d[:])
            ones128 = aux[:, 0:128]
            ones64 = aux[:, 0:64]
            onescol = cst.tile([128, MT1], F32R, tag="onescol")
            nc.sync.dma_start(onescol[:], ones_col_d[:])
            b1_sb, b2qk_sb = {}, {}
            for t in "kqv":
                b1_sb[t] = cst.tile([128, MT1], F32R, tag=f"b1{t}", name=f"b1sb{t}")
                nc.sync.dma_start(b1_sb[t][:],
                                  b1[t].ap().rearrange("(m p) -> p m", p=128))
            for t in "kq":
                b2qk_sb[t] = cst.tile([128, MT2], F32, tag=f"b2{t}", name=f"b2sb{t}")
                nc.sync.dma_start(b2qk_sb[t][:],
                                  b2[t].ap().rearrange("(m p) -> p m", p=128))
            b2v_sb = aux[:, 128:128 + D]
            nc.sync.dma_start(b2v_sb, b2["v"].ap().rearrange("(a d) -> a d", a=1))

            def mlp(t, transposed):
                w1_t = []
                for kt in range(KT1):
                    w = w1p.tile([128, H], F32R, tag="w1", name=f"w1_{t}{kt}")
                    nc.sync.dma_start(w[:], w1[t][kt * 128:(kt + 1) * 128, :])
                    w1_t.append(w)
                w2_t = []
                for kt in range(MT1):
                    w = w2p.tile([128, D], F32R, tag="w2", name=f"w2_{t}{kt}")
                    nc.sync.dma_start(w[:], w2[t][kt * 128:(kt + 1) * 128, :])
                    w2_t.append(w)
                h1_t = []
                for m in range(MT1):
                    pp = psp.tile([128, ROWS], F32, tag="ps", name=f"ps1_{t}{m}")
                    for kt in range(KT1):
                        nc.tensor.matmul(pp[:],
                                         w1_t[kt][:, m * 128:(m + 1) * 128],
                                         xt[:, kt * ROWS:(kt + 1) * ROWS],
                                         start=(kt == 0), stop=(kt == KT1 - 1))
                    h1 = h1p.tile([128, ROWS], F32R, tag="h1", name=f"h1_{t}{m}")
                    nc.scalar.activation(h1[:], pp[:], AF.Gelu_apprx_tanh,
                                         bias=b1_sb[t][:, m:m + 1])
                    h1_t.append(h1)
                if transposed:          # k, q: out = W2^T h1T + b2  [D, ROWS]
                    for m in range(MT2):
                        pp = psp.tile([128, ROWS], F32, tag="ps", name=f"ps2_{t}{m}")
                        for kt in range(MT1):
                            nc.tensor.matmul(pp[:],
                                             w2_t[kt][:, m * 128:(m + 1) * 128],
                                             h1_t[kt][:],
                                             start=(kt == 0), stop=(kt == MT1 - 1))
                        ot = l2p.tile([128, ROWS], F32R, tag="l2", name=f"l2_{t}{m}")
                        nc.vector.tensor_scalar_add(ot[:], pp[:],
                                                    b2qk_sb[t][:, m:m + 1])
                        nc.sync.dma_start(send[t][m * 128:(m + 1) * 128, :], ot[:])
                        if t == "k":
                            nc.sync.dma_start(kT_out[m * 128:(m + 1) * 128, :],
                                              ot[:])
                else:                   # v: out = gelu(h1 W2 + b2)  [ROWS, D]
                    for m in range(MT2):
                        pp = psp.tile([128, D], F32, tag="ps", name=f"ps2_{t}{m}")
                        for kt in range(MT1):
                            nc.tensor.matmul(pp[:],
                                             h1_t[kt][:, m * 128:(m + 1) * 128],
                                             w2_t[kt][:],
                                             start=(kt == 0), stop=False)
                        nc.tensor.matmul(pp[:], ones128, b2v_sb,
                                         start=False, stop=True)
                        ot = l2p.tile([128, D], F32R, tag="l2", name=f"l2_{t}{m}")
                        nc.scalar.activation(ot[:], pp[:], AF.Gelu_apprx_tanh)
                        nc.sync.dma_start(v_out[m * 128:(m + 1) * 128, :], ot[:])
                        nc.sync.dma_start(
                            send["v"][:, m * 128:(m + 1) * 128, :]
                            .rearrange("p r d -> r p d"),
                            ot[:].rearrange("r (p d) -> r p d", p=N_CORES))

            def a2a(t):
                nc.gpsimd.collective_compute(
                    "AllToAll", mybir.AluOpType.bypass,
                    replica_groups=[list(range(N_CORES))],
                    ins=[send[t].opt()], outs=[recv[t].opt()])

            mlp("k", True)
            a2a("k")
            mlp("q", True)
            a2a("q")
            mlp("v", False)
            a2a("v")

            # ---- attention: head c (this core), per batch ----
            for b in range(N):
                qT_sb = attp.tile([Dh, S], F32R, tag="qT", name=f"qT{b}")
                kT_sb = attp.tile([Dh, S], F32R, tag="kT", name=f"kT{b}")
                vaug = attp.tile([128, MT1 * 65], F32R, tag="vaug", name=f"va{b}")
                for j in range(N_CORES):
                    nc.sync.dma_start(
                        qT_sb[:, j * RPC:(j + 1) * RPC],
                        recv["q"][j * Dh:(j + 1) * Dh, b * RPC:(b + 1) * RPC])
                    nc.sync.dma_start(
                        kT_sb[:, j * RPC:(j + 1) * RPC],
                        recv["k"][j * Dh:(j + 1) * Dh, b * RPC:(b + 1) * RPC])
                nc.sync.dma_start(
                    vaug[:].rearrange("p (g c) -> p g c", c=65)[:, :, 64:65],
                    onescol[:].rearrange("p (g o) -> p g o", o=1))
                for g in range(MT1):
                    j, half = g // 2, g % 2
                    r0 = b * RPC + half * 128
                    nc.sync.dma_start(vaug[:, g * 65:g * 65 + 64],
                                      recv["v"][j, r0:r0 + 128, :])
                for qc in range(QC):
                    q0 = qc * 512
                    nk = 4 * qc + 4
                    exps = [None] * nk
                    po = psop.tile([65, 512], F32, tag="pso", name=f"po{b}{qc}")

                    def scores(kt):
                        pp = psp.tile([128, 512], F32, tag="ps",
                                      name=f"sc{b}{qc}_{kt}")
                        nc.tensor.matmul(pp[:],
                                         kT_sb[:, kt * 128:(kt + 1) * 128],
                                         qT_sb[:, q0:q0 + 512],
                                         start=True, stop=True)
                        ex = expp.tile([128, 512], F32R, tag="exp",
                                       name=f"ex{b}{qc}_{kt}")
                        nc.scalar.activation(ex[:], pp[:], AF.Exp, scale=0.125)
                        o = kt * 128 - q0
                        if o >= 0:      # diagonal tile: 0/1 causal band mask
                            nc.vector.tensor_mul(ex[:], ex[:],
                                                 band_sb[:, 512 - o:1024 - o])
                        exps[kt] = ex

                    def pv(kt):
                        nc.tensor.matmul(po[:],
                                         vaug[:, kt * 65:(kt + 1) * 65],
                                         exps[kt][:],
                                         start=(kt == 0), stop=(kt == nk - 1))

                    # software-pipeline: PV lags scores by 2 tiles
                    for kt in range(nk):
                        scores(kt)
                        if kt >= 2:
                            pv(kt - 2)
                    pv(nk - 2)
                    pv(nk - 1)

                    r_sb = smp.tile([1, 512], F32R, tag="r", name=f"r{b}{qc}")
                    with nc.allow_low_precision(reason="f32r same width as f32"):
                        nc.vector.reciprocal(r_sb[:], po[64:65, :])
                    pb = psbp.tile([64, 512], F32, tag="psb", name=f"pb{b}{qc}")
                    nc.tensor.matmul(pb[:], ones64, r_sb[:],
                                     start=True, stop=True)
                    rb = smp.tile([64, 512], F32R, tag="rb", name=f"rb{b}{qc}")
                    nc.vector.tensor_copy(rb[:], pb[:])
                    oT = smp.tile([64, 512], F32R, tag="oT", name=f"oT{b}{qc}")
                    nc.vector.tensor_mul(oT[:], po[0:64, :], rb[:])
                    nc.sync.dma_start(aT_out[b, :, q0:q0 + 512], oT[:])

    nc.compile()
    return nc


_COMPILED = None


def _get_compiled():
    global _COMPILED
    if _COMPILED is None:
        _COMPILED = _build()
    return _COMPILED


def _band_mask():
    return (np.arange(1024, dtype=np.int32)[None, :]
            >= (np.arange(128, dtype=np.int32)[:, None] + 512)).astype(np.float32)


def _f32(a):
    return np.ascontiguousarray(np.asarray(a, dtype=np.float32))


def _make_in_maps(x, qW1, qb1, qW2, qb2, kW1, kb1, kW2, kb2, vW1, vb1,
                  vW2, vb2):
    x = _f32(x)
    band = _band_mask()
    shared = {
        "w1q": _f32(qW1), "w1k": _f32(kW1), "w1v": _f32(vW1),
        "w2q": _f32(qW2), "w2k": _f32(kW2), "w2v": _f32(vW2),
        "b1q": _f32(qb1), "b1k": _f32(kb1), "b1v": _f32(vb1),
        "b2q": _f32(qb2), "b2k": _f32(kb2), "b2v": _f32(vb2),
        "band": band,
        "ones_col": np.ones((128, MT1), np.float32),
        "ones_row": np.ones((1, 128), np.float32),
    }
    in_maps = []
    for c in range(N_CORES):
        xc = np.concatenate([x[b, c * RPC:(c + 1) * RPC, :] for b in range(N)], 0)
        im = dict(shared)
        im["xT"] = np.ascontiguousarray(xc.T)
        in_maps.append(im)
    return in_maps


def _assemble(res):

    k_full = np.empty((N, S, D), np.float32)
    v_full = np.empty((N, S, D), np.float32)
    out_full = np.empty((N, S, D), np.float32)
    for j in range(N_CORES):
        kT_j = res[j]["kT_out"]          # [D, ROWS]
        v_j = res[j]["v_out"]            # [ROWS, D]
        aT_j = res[j]["attn_outT"]       # [N, Dh, S]
        for b in range(N):
            k_full[b, j * RPC:(j + 1) * RPC, :] = kT_j[:, b * RPC:(b + 1) * RPC].T
            v_full[b, j * RPC:(j + 1) * RPC, :] = v_j[b * RPC:(b + 1) * RPC, :]
            out_full[b, :, j * Dh:(j + 1) * Dh] = aT_j[b].T
    return k_full, v_full, out_full


def kernel(**inputs):
    nc = _get_compiled()
    in_maps = _make_in_maps(**inputs)
    res = run_bass_kernel_spmd(nc, in_maps, list(range(N_CORES))).results
    return _assemble(res)
